# revision 21
# baseline (speedup 1.0000x reference)
"""BERT self-attention (B=8, S=1024, H=1024, 16 heads) on 8 TRN2 NeuronCores.

Sharding: data-parallel over batch — core i computes batch element i end to
end (QKV projections + attention), no collectives.

v2 structure (fastest measured) with the two-point Schraudolph swap:
  CPU pre-pack: X^T, Wq, Wk, Wv are repacked to bf16 in SBUF feed layout
               ([p, kt, s] / [p, kt, m]); four big contiguous DMAs per rep,
               zero on-device transposes.
  V          = X @ Wv scattered to per-head slices + ones column
  Q^T, K^T   = Wq/Wk col-block matmuls against X^T (layout [H', S]),
               all head pairs projected up front
  scores^T   = K_h @ Q_h^T per head ([key, query] layout, contraction=64)
  P          = exp(scores^T/8 + mask): ScalarE exact exp for 96 tiles;
               DVE two-point-average Schraudolph (sawtooth-cancelled bit
               trick, ±1.2%) for 32 tiles
  ctx'^T     = V'^T @ P  -> rows 0..63 = ctx^T, row 64 = softmax denominator
  ctx        = batched PE transposes + one batched reciprocal + one
               broadcast (stride-0) multiply per head
"""

import numpy as np

import concourse.bass as bass
import concourse.mybir as mybir
import concourse.tile as tile
from concourse.bass_utils import run_bass_kernel_spmd
from concourse.masks import make_identity
from concourse.vector_clock import ScopedClock

S = 1024
H = 1024
NH = 16
HD = 64
P = 128
NS = S // P  # s tiles
NK = H // P  # contraction tiles
NHP = NH // 2  # head pairs (one 128-partition tile of Q^T/K^T per pair)
VW = HD + 1  # V' width per head (extra ones column)
F32 = mybir.dt.float32
F32R = mybir.dt.float32r
BF16 = mybir.dt.bfloat16
I16 = mybir.dt.int16
N_CORES = 8

# Two-point-average Schraudolph fast-exp on DVE (bf16 bit trick, sawtooth
# cancelled by averaging two evaluations half a mantissa period apart):
#   i16_a = round(s*SCH_A + B1),  i16_b = i16_a - 64  (exact, int domain)
#   P     = bf16(bitcast_bf16(i16_b) * sqrt(2) + bitcast_bf16(i16_a))
# B1 = 128*(127 - sigma) - 128 (the -128 folds the x0.5 of the average);
# the additive mask lands in B via SCH_A*8*mask. Max rel err ~1.2%.
SCH_A = float(16 * np.log2(np.e))
SCH_B1 = 128.0 * (127.0 - 0.0548) - 128.0
SQRT2 = float(np.sqrt(2.0))
DVE_KTS = (0, 2, 4, 6)

_DRAIN_WAIT_CHUNK = 1
_patched = False


def _patch_tile_drain():
    """The walrus build in this container rejects instructions carrying more
    than a few sem waits; TileContext's tail drain waits on every live sem in
    one instruction. Split it into one drain per wait."""
    global _patched
    if _patched:
        return
    _patched = True

    def _drain_and_barrier(self, tick_clock, wait_clock):
        drain_inst = self.nc.sync.drain()
        wait_clock.add_sem_waits(
            drain_inst.ins, ScopedClock({None: tick_clock.global_clock})
        )
        si = drain_inst.ins.sync_info
        if si is not None and len(si.on_wait) > _DRAIN_WAIT_CHUNK:
            waits = list(si.on_wait)
            si.on_wait = waits[:_DRAIN_WAIT_CHUNK]
            drain_inst.ins.sync_info = si
            for i in range(_DRAIN_WAIT_CHUNK, len(waits), _DRAIN_WAIT_CHUNK):
                extra = self.nc.sync.drain()
                extra.ins.sync_info = mybir.SyncInfo(
                    on_wait=waits[i : i + _DRAIN_WAIT_CHUNK], on_update=[]
                )
        self.nc.all_engine_barrier()
        popped = self.nc._tile_sem_poison_stack.pop()
        assert popped is self._sem_poison
        self.nc.clear_and_free_semaphores(list(self.sems.allocated().values()))
        self.nc.all_engine_barrier()

    tile.TileContext._drain_and_barrier = _drain_and_barrier


def build_consts(nc, tc, ctx, mask, bq, bk, bv):
    """Constant tiles shared by all reps: identity, mask/bias layouts, ones."""
    singles = ctx.enter_context(tc.tile_pool(name="singles", bufs=1))
    ident = singles.tile([P, P], F32)
    make_identity(nc, ident)
    mask_t = singles.tile([P, NK], F32)
    nc.sync.dma_start(out=mask_t, in_=mask.rearrange("(t p) -> p t", p=P))
    # Schraudolph bias per key-partition: B1 + SCH_A*8*mask
    schb_t = singles.tile([P, NK], F32)
    nc.vector.tensor_scalar(
        out=schb_t,
        in0=mask_t,
        scalar1=float(SCH_A * 8.0),
        scalar2=float(SCH_B1),
        op0=mybir.AluOpType.mult,
        op1=mybir.AluOpType.add,
    )
    bq_t = singles.tile([P, NHP], F32)
    nc.sync.dma_start(out=bq_t, in_=bq.rearrange("(t p) -> p t", p=P))
    bk_t = singles.tile([P, NHP], F32)
    nc.sync.dma_start(out=bk_t, in_=bk.rearrange("(t p) -> p t", p=P))
    ones_col = singles.tile([P, NH, 1], BF16)
    nc.vector.memset(ones_col, 1.0)
    bv_b = singles.tile([P, H], F32)
    nc.gpsimd.dma_start(
        out=bv_b,
        in_=bass.AP(tensor=bv.tensor, offset=bv.offset, ap=[[0, P], bv.ap[0]]),
    )
    return ident, mask_t, schb_t, bq_t, bk_t, ones_col, bv_b


def build_kernel(nc, tc, consts, hst, mask, wqf, bq, wkf, bk, wvf, bv, out,
                 phases=("v", "qk", "attn"), dve_kts=DVE_KTS):
    from contextlib import ExitStack

    Exp = mybir.ActivationFunctionType.Exp
    ident, mask_t, schb_t, bq_t, bk_t, ones_col, bv_b = consts

    ctx = ExitStack()
    with ctx:
        xt_pool = ctx.enter_context(tc.tile_pool(name="xt", bufs=1))
        w_pool = ctx.enter_context(tc.tile_pool(name="wf", bufs=1))
        vp_pool = ctx.enter_context(tc.tile_pool(name="vp", bufs=NS))
        qk_pool = ctx.enter_context(tc.tile_pool(name="qk", bufs=NHP))
        outp_pool = ctx.enter_context(tc.tile_pool(name="outp", bufs=2))
        ctx_pool = ctx.enter_context(tc.tile_pool(name="ctxp", bufs=2))
        rec_pool = ctx.enter_context(tc.tile_pool(name="rec", bufs=2))
        # PSUM budget (8 banks): sps 2x[128,1024]=4 (per-head scores chunks,
        # also hosts the projection accumulators via the same tag), cps
        # 2x[128,1024]=4 (per-head PV accums; the batched ctx-transpose tiles
        # reuse the same ring)
        sps_pool = ctx.enter_context(tc.tile_pool(name="spsp", bufs=2, space="PSUM"))
        cps_pool = ctx.enter_context(tc.tile_pool(name="cpsp", bufs=2, space="PSUM"))

        # ---- phase 0: load X^T and weights (pre-packed bf16, contiguous) ----
        xt = xt_pool.tile([P, NK, S], BF16, tag="xt")
        nc.sync.dma_start(out=xt, in_=hst)
        wv_t = w_pool.tile([P, NK, H], BF16, tag="wv")
        nc.scalar.dma_start(out=wv_t, in_=wvf)
        wq_t = w_pool.tile([P, NK, H], BF16, tag="wq")
        nc.gpsimd.dma_start(out=wq_t, in_=wqf)
        wk_t = w_pool.tile([P, NK, H], BF16, tag="wk")
        nc.gpsimd.dma_start(out=wk_t, in_=wkf)

        vp = [vp_pool.tile([P, NH * VW], BF16, tag="vp", name=f"vp{i}") for i in range(NS)]

        # ---- phase v: V = X @ Wv, scattered per head with ones column ----
        if "v" in phases:
            for st in range(NS):
                v3d = vp[st].rearrange("p (h c) -> p h c", c=VW)
                nc.vector.tensor_copy(v3d[:, :, HD : HD + 1], ones_col)
                vps = sps_pool.tile([P, S], F32, tag="sps", name="vps")
                for kt in range(NK):
                    for hc in range(2):
                        nc.tensor.matmul(
                            vps[:, hc * 512 : (hc + 1) * 512],
                            lhsT=xt[:, kt, st * P : (st + 1) * P],
                            rhs=wv_t[:, kt, hc * 512 : (hc + 1) * 512],
                            start=(kt == 0),
                            stop=(kt == NK - 1),
                        )
                nc.vector.tensor_add(
                    v3d[:, :, 0:HD],
                    vps.rearrange("p (h c) -> p h c", c=HD),
                    bv_b.rearrange("p (h c) -> p h c", c=HD),
                )

        # ---- per head pair: project Q^T/K^T, then attention for both heads ----
        def emit_proj(hp):
            """Q^T/K^T projection for head pair hp: PE accumulation through
            the sps ring + bias add onto SBUF (bf16)."""
            qt_t = qk_pool.tile([P, S], BF16, tag="qt")
            qps = sps_pool.tile([P, S], F32, tag="sps", name="qps")
            for kt in range(NK):
                for sc in range(2):
                    nc.tensor.matmul(
                        qps[:, sc * 512 : (sc + 1) * 512],
                        lhsT=wq_t[:, kt, hp * P : (hp + 1) * P],
                        rhs=xt[:, kt, sc * 512 : (sc + 1) * 512],
                        start=(kt == 0),
                        stop=(kt == NK - 1),
                    )
            nc.vector.tensor_scalar_add(qt_t, qps, bq_t[:, hp : hp + 1])

            kt_t = qk_pool.tile([P, S], BF16, tag="kt")
            kps = sps_pool.tile([P, S], F32, tag="sps", name="kps")
            for kt in range(NK):
                for sc in range(2):
                    nc.tensor.matmul(
                        kps[:, sc * 512 : (sc + 1) * 512],
                        lhsT=wk_t[:, kt, hp * P : (hp + 1) * P],
                        rhs=xt[:, kt, sc * 512 : (sc + 1) * 512],
                        start=(kt == 0),
                        stop=(kt == NK - 1),
                    )
            nc.vector.tensor_scalar_add(kt_t, kps, bk_t[:, hp : hp + 1])
            return qt_t, kt_t

        # Project Q^T/K^T for ALL head pairs up front (a pure PE GEMM streak).
        # The attention loop's PSUM ring then only serves scores, so
        # scores(hp+1) start right after exp(hp) drains and the exp engines
        # stay hot across head-pair boundaries.
        with (
            tc.tile_pool(name="expt", bufs=3) as exp_pool,
            tc.tile_pool(name="scht", bufs=2) as sch_pool,
        ):
            n_hp = NHP if "qk" in phases else 0
            qk_all = [emit_proj(h) for h in range(n_hp)]
            for hp in range(n_hp):
                qt_t, kt_t = qk_all[hp]

                outp = outp_pool.tile([P, S], F32, tag="outp")
                a_on = "attn" in phases
                exp2 = [
                    exp_pool.tile([P, NK * S], BF16, tag="expt", name=f"exp{i}")
                    for i in range(2)
                ]
                cps2 = [
                    cps_pool.tile([P, 1024], F32, tag="cps", name=f"cps{i}")
                    for i in range(2)
                ]
                for kt in range(NK if a_on else 0):
                    sp2 = [
                        sps_pool.tile([P, S], F32, tag="sps", name=f"sp{i}")
                        for i in range(2)
                    ]
                    # sub-outer: each head's K^T block stays stationary for
                    # both query chunks (one weight load per head per kt);
                    # subs still land on distinct row groups / PSUM banks so
                    # the PE can overlap them
                    for sub in range(2):
                        r0 = HD * sub
                        for qn in range(2):
                            nc.tensor.matmul(
                                sp2[sub][:, qn * 512 : (qn + 1) * 512],
                                lhsT=kt_t[r0 : r0 + HD, kt * P : (kt + 1) * P],
                                rhs=qt_t[r0 : r0 + HD, qn * 512 : (qn + 1) * 512],
                                start=True,
                                stop=True,
                            )
                    for sub in range(2):
                        if sub == 1 and kt in dve_kts:
                            # two-point Schraudolph on DVE (see header)
                            tma = sch_pool.tile([P, S], BF16, tag="scha")
                            tmb = sch_pool.tile([P, S], BF16, tag="schb")
                            nc.vector.tensor_scalar(
                                out=tma.bitcast(I16),
                                in0=sp2[sub],
                                scalar1=SCH_A,
                                scalar2=schb_t[:, kt : kt + 1],
                                op0=mybir.AluOpType.mult,
                                op1=mybir.AluOpType.add,
                            )
                            # i16_b = i16_a - 64 exactly (int domain, 4x mode)
                            nc.vector.tensor_scalar_add(
                                tmb.bitcast(I16), tma.bitcast(I16), -64
                            )
                            # combine via two 4x-eligible bf16 ops (the
                            # one-shot STT runs at 1x on DVE)
                            nc.vector.tensor_scalar_mul(tmb, tmb, SQRT2)
                            nc.vector.tensor_add(
                                exp2[sub][:, kt * S : (kt + 1) * S], tma, tmb
                            )
                        else:
                            nc.scalar.activation(
                                exp2[sub][:, kt * S : (kt + 1) * S],
                                sp2[sub],
                                Exp,
                                bias=mask_t[:, kt : kt + 1],
                                scale=1.0 / np.sqrt(HD),
                            )
                    for sub in range(2):
                        h = 2 * hp + sub
                        for qn in range(2):
                            nc.tensor.matmul(
                                cps2[sub][:VW, qn * 512 : (qn + 1) * 512],
                                lhsT=vp[kt][:, h * VW : (h + 1) * VW],
                                rhs=exp2[sub][
                                    :, kt * S + qn * 512 : kt * S + qn * 512 + 512
                                ],
                                start=(kt == 0),
                                stop=(kt == NK - 1),
                            )
                for sub in range(2 if a_on else 0):
                    r0 = HD * sub
                    ctx_sb = ctx_pool.tile([VW, S], F32, tag="ctx")
                    nc.vector.tensor_copy(ctx_sb, cps2[sub][:VW, :])
                    # batched transposes into one padded PSUM tile (per-qt
                    # stride 128 keeps each transpose inside one bank)
                    tpb = cps_pool.tile([P, NS, P], F32, tag="cps", name="tpb")
                    for qt in range(NS):
                        nc.tensor.transpose(
                            tpb[:, qt, :VW],
                            ctx_sb[:, qt * P : (qt + 1) * P],
                            ident[:VW, :VW],
                        )
                    # one batched reciprocal over all 8 denominator columns,
                    # then one broadcast multiply scattering to output layout
                    rec = rec_pool.tile([P, NS], F32, tag="rec")
                    nc.vector.reciprocal(rec, tpb[:, :, HD])
                    rec_b = bass.AP(
                        tensor=rec.tensor,
                        offset=rec.offset,
                        ap=[rec.ap[0], rec.ap[1], [0, HD]],
                    )
                    nc.vector.tensor_mul(
                        outp.rearrange("p (q c) -> p q c", c=P)[:, :, r0 : r0 + HD],
                        tpb[:, :, 0:HD],
                        rec_b,
                    )
                if "attn" in phases:
                    # issue on the idle GPSIMD DGE: these wait on the whole
                    # attention pipeline, and on the in-order SP queue they
                    # would block the next rep's input loads from issuing
                    nc.gpsimd.dma_start(
                        out=out.rearrange("(q p) c -> p q c", p=P)[
                            :, :, hp * P : (hp + 1) * P
                        ],
                        in_=outp.rearrange("p (q c) -> p q c", c=P),
                    )


def _split_excess_waits(nc):
    """This walrus build rejects instructions with more than a couple of sem
    waits. Hoist excess waits onto injected same-engine NoOps that execute
    immediately before the overfull instruction (program order per engine is
    the basic-block order, so the waits still complete first)."""
    counter = 0
    for func in nc.m.functions:
        for block in func.blocks:
            insts = block.instructions
            out = []
            changed = False
            for inst in insts:
                si = inst.sync_info
                limit = 2 if type(inst).__name__ == "InstEventSemaphore" else 1
                if si is not None and len(si.on_wait) > limit:
                    waits = list(si.on_wait)
                    for w in waits[limit:]:
                        nop = mybir.InstNoOp(
                            name=f"I-wsplit-{counter}", engine=inst.engine
                        )
                        counter += 1
                        nop.sync_info = mybir.SyncInfo(on_wait=[w], on_update=[])
                        nop.debug = inst.debug
                        out.append(nop)
                    si.on_wait = waits[:limit]
                    inst.sync_info = si
                    changed = True
                out.append(inst)
            if changed:
                block.instructions = out


_NC_CACHE = {}


def _build(split_waits=True, n_reps=1, phases=("v", "qk", "attn"), **kw):
    global _NC_CACHE
    key = (n_reps, tuple(phases), tuple(sorted(kw.items())))
    if split_waits and key in _NC_CACHE:
        return _NC_CACHE[key]
    _patch_tile_drain()
    nc = bass.Bass(target_bir_lowering=False, debug=False)
    hst = nc.dram_tensor("hst", [P, NK, S], BF16, kind="ExternalInput").ap()
    mask = nc.dram_tensor("mask", [S], F32, kind="ExternalInput").ap()
    wqf = nc.dram_tensor("wqf", [P, NK, H], BF16, kind="ExternalInput").ap()
    bq = nc.dram_tensor("bq", [H], F32, kind="ExternalInput").ap()
    wkf = nc.dram_tensor("wkf", [P, NK, H], BF16, kind="ExternalInput").ap()
    bk = nc.dram_tensor("bk", [H], F32, kind="ExternalInput").ap()
    wvf = nc.dram_tensor("wvf", [P, NK, H], BF16, kind="ExternalInput").ap()
    bv = nc.dram_tensor("bv", [H], F32, kind="ExternalInput").ap()
    out = nc.dram_tensor("out", [S, H], F32, kind="ExternalOutput").ap()
    from contextlib import ExitStack

    with tile.TileContext(nc) as tc, ExitStack() as cctx:
        consts = build_consts(nc, tc, cctx, mask, bq, bk, bv)
        for _ in range(n_reps):
            build_kernel(
                nc, tc, consts, hst, mask, wqf, bq, wkf, bk, wvf, bv, out,
                phases=phases, **kw
            )
    if not split_waits:
        return nc
    _split_excess_waits(nc)
    _NC_CACHE[key] = nc
    return nc


def make_in_maps(hidden_states, attention_mask, Wq, bq, Wk, bk, Wv, bv):
    import ml_dtypes

    bf16 = ml_dtypes.bfloat16
    f = np.ascontiguousarray

    def feed(w):
        # [H, H] -> [P, NK, H] with w[t*P+p, m] at [p, t, m]
        return f(
            np.asarray(w, dtype=np.float32)
            .reshape(NK, P, H)
            .transpose(1, 0, 2)
            .astype(bf16)
        )

    wqf, wkf, wvf = feed(Wq), feed(Wk), feed(Wv)
    maps = []
    for i in range(N_CORES):
        x = np.asarray(hidden_states[i], dtype=np.float32)
        # X^T feed: [P, NK, S] with X[s, k*P+p] at [p, k, s]
        hst = f(x.reshape(S, NK, P).transpose(2, 1, 0).astype(bf16))
        maps.append(
            {
                "hst": hst,
                "mask": f(
                    np.asarray(attention_mask[i], dtype=np.float32).reshape(S)
                ),
                "wqf": wqf,
                "bq": f(np.asarray(bq, dtype=np.float32)),
                "wkf": wkf,
                "bk": f(np.asarray(bk, dtype=np.float32)),
                "wvf": wvf,
                "bv": f(np.asarray(bv, dtype=np.float32)),
            }
        )
    return maps


def run(in_maps, **kwargs):
    nc = _build()
    return run_bass_kernel_spmd(nc, in_maps, core_ids=list(range(N_CORES)), **kwargs)


def kernel(hidden_states, attention_mask, Wq, bq, Wk, bk, Wv, bv):
    in_maps = make_in_maps(hidden_states, attention_mask, Wq, bq, Wk, bk, Wv, bv)
    res = run(in_maps)
    return np.stack([res.results[i]["out"] for i in range(N_CORES)], axis=0)


# revision 23
# speedup vs baseline: 1.0543x; 1.0543x over previous
"""BERT self-attention (B=8, S=1024, H=1024, 16 heads) on 8 TRN2 NeuronCores.

Sharding: data-parallel over batch — core i computes batch element i end to
end (QKV projections + attention), no collectives.

v2 structure (fastest measured) with the two-point Schraudolph swap:
  CPU pre-pack: X^T, Wq, Wk, Wv are repacked to bf16 in SBUF feed layout
               ([p, kt, s] / [p, kt, m]); four big contiguous DMAs per rep,
               zero on-device transposes.
  V          = X @ Wv scattered to per-head slices + ones column
  Q^T, K^T   = Wq/Wk col-block matmuls against X^T (layout [H', S]),
               all head pairs projected up front
  scores^T   = K_h @ Q_h^T per head ([key, query] layout, contraction=64)
  P          = exp(scores^T/8 + mask): ScalarE exact exp for 96 tiles;
               DVE two-point-average Schraudolph (sawtooth-cancelled bit
               trick, ±1.2%) for 32 tiles
  ctx'^T     = V'^T @ P  -> rows 0..63 = ctx^T, row 64 = softmax denominator
  ctx        = batched PE transposes + one batched reciprocal + one
               broadcast (stride-0) multiply per head
"""

import numpy as np

import concourse.bass as bass
import concourse.mybir as mybir
import concourse.tile as tile
from concourse.bass_utils import run_bass_kernel_spmd
from concourse.masks import make_identity
from concourse.vector_clock import ScopedClock

S = 1024
H = 1024
NH = 16
HD = 64
P = 128
NS = S // P  # s tiles
NK = H // P  # contraction tiles
NHP = NH // 2  # head pairs (one 128-partition tile of Q^T/K^T per pair)
VW = HD + 1  # V' width per head (extra ones column)
F32 = mybir.dt.float32
F32R = mybir.dt.float32r
BF16 = mybir.dt.bfloat16
I16 = mybir.dt.int16
N_CORES = 8

# Two-point-average Schraudolph fast-exp on DVE (bf16 bit trick, sawtooth
# cancelled by averaging two evaluations half a mantissa period apart):
#   i16_a = round(s*SCH_A + B1),  i16_b = i16_a - 64  (exact, int domain)
#   P     = bf16(bitcast_bf16(i16_b) * sqrt(2) + bitcast_bf16(i16_a))
# B1 = 128*(127 - sigma) - 128 (the -128 folds the x0.5 of the average);
# the additive mask lands in B via SCH_A*8*mask. Max rel err ~1.2%.
SCH_A = float(16 * np.log2(np.e))
SCH_B1 = 128.0 * (127.0 - 0.0548) - 128.0
SQRT2 = float(np.sqrt(2.0))
DVE_KTS = (0, 2, 4, 6)

_DRAIN_WAIT_CHUNK = 1
_patched = False


def _patch_tile_drain():
    """The walrus build in this container rejects instructions carrying more
    than a few sem waits; TileContext's tail drain waits on every live sem in
    one instruction. Split it into one drain per wait."""
    global _patched
    if _patched:
        return
    _patched = True

    def _drain_and_barrier(self, tick_clock, wait_clock):
        drain_inst = self.nc.sync.drain()
        wait_clock.add_sem_waits(
            drain_inst.ins, ScopedClock({None: tick_clock.global_clock})
        )
        si = drain_inst.ins.sync_info
        if si is not None and len(si.on_wait) > _DRAIN_WAIT_CHUNK:
            waits = list(si.on_wait)
            si.on_wait = waits[:_DRAIN_WAIT_CHUNK]
            drain_inst.ins.sync_info = si
            for i in range(_DRAIN_WAIT_CHUNK, len(waits), _DRAIN_WAIT_CHUNK):
                extra = self.nc.sync.drain()
                extra.ins.sync_info = mybir.SyncInfo(
                    on_wait=waits[i : i + _DRAIN_WAIT_CHUNK], on_update=[]
                )
        self.nc.all_engine_barrier()
        popped = self.nc._tile_sem_poison_stack.pop()
        assert popped is self._sem_poison
        self.nc.clear_and_free_semaphores(list(self.sems.allocated().values()))
        self.nc.all_engine_barrier()

    tile.TileContext._drain_and_barrier = _drain_and_barrier


def build_consts(nc, tc, ctx, mask, bq, bk, bv):
    """Constant tiles shared by all reps: identity, mask/bias layouts, ones."""
    singles = ctx.enter_context(tc.tile_pool(name="singles", bufs=1))
    ident = singles.tile([P, P], F32)
    make_identity(nc, ident)
    mask_t = singles.tile([P, NK], F32)
    nc.sync.dma_start(out=mask_t, in_=mask.rearrange("(t p) -> p t", p=P))
    # Schraudolph bias per key-partition: B1 + SCH_A*8*mask
    schb_t = singles.tile([P, NK], F32)
    nc.vector.tensor_scalar(
        out=schb_t,
        in0=mask_t,
        scalar1=float(SCH_A * 8.0),
        scalar2=float(SCH_B1),
        op0=mybir.AluOpType.mult,
        op1=mybir.AluOpType.add,
    )
    bq_t = singles.tile([P, NHP], F32)
    nc.sync.dma_start(out=bq_t, in_=bq.rearrange("(t p) -> p t", p=P))
    bk_t = singles.tile([P, NHP], F32)
    nc.sync.dma_start(out=bk_t, in_=bk.rearrange("(t p) -> p t", p=P))
    ones_col = singles.tile([P, NH, 1], BF16)
    nc.vector.memset(ones_col, 1.0)
    bv_b = singles.tile([P, H], F32)
    nc.gpsimd.dma_start(
        out=bv_b,
        in_=bass.AP(tensor=bv.tensor, offset=bv.offset, ap=[[0, P], bv.ap[0]]),
    )
    return ident, mask_t, schb_t, bq_t, bk_t, ones_col, bv_b


def build_kernel(nc, tc, consts, hst, mask, wqf, bq, wkf, bk, wvf, bv, out,
                 phases=("v", "qk", "attn"), dve_kts=DVE_KTS):
    from contextlib import ExitStack

    Exp = mybir.ActivationFunctionType.Exp
    ident, mask_t, schb_t, bq_t, bk_t, ones_col, bv_b = consts

    ctx = ExitStack()
    with ctx:
        xt_pool = ctx.enter_context(tc.tile_pool(name="xt", bufs=1))
        w_pool = ctx.enter_context(tc.tile_pool(name="wf", bufs=1))
        vp_pool = ctx.enter_context(tc.tile_pool(name="vp", bufs=NS))
        qk_pool = ctx.enter_context(tc.tile_pool(name="qk", bufs=NHP))
        outp_pool = ctx.enter_context(tc.tile_pool(name="outp", bufs=2))
        ctx_pool = ctx.enter_context(tc.tile_pool(name="ctxp", bufs=2))
        rec_pool = ctx.enter_context(tc.tile_pool(name="rec", bufs=2))
        # PSUM budget (8 banks): sps 2x[128,1024]=4 (per-head scores chunks,
        # also hosts the projection accumulators via the same tag), cps
        # 2x[128,1024]=4 (per-head PV accums; the batched ctx-transpose tiles
        # reuse the same ring)
        sps_pool = ctx.enter_context(tc.tile_pool(name="spsp", bufs=2, space="PSUM"))
        cps_pool = ctx.enter_context(tc.tile_pool(name="cpsp", bufs=2, space="PSUM"))

        # ---- phase 0: load X^T and weights (pre-packed bf16, contiguous) ----
        xt = xt_pool.tile([P, NK, S], BF16, tag="xt")
        nc.sync.dma_start(out=xt, in_=hst)
        wv_t = w_pool.tile([P, NK, H], BF16, tag="wv")
        nc.scalar.dma_start(out=wv_t, in_=wvf)
        wq_t = w_pool.tile([P, NK, H], BF16, tag="wq")
        nc.gpsimd.dma_start(out=wq_t, in_=wqf)
        wk_t = w_pool.tile([P, NK, H], BF16, tag="wk")
        nc.gpsimd.dma_start(out=wk_t, in_=wkf)

        vp = [vp_pool.tile([P, NH * VW], BF16, tag="vp", name=f"vp{i}") for i in range(NS)]

        # ---- phase v: V = X @ Wv, scattered per head with ones column ----
        if "v" in phases:
            for st in range(NS):
                v3d = vp[st].rearrange("p (h c) -> p h c", c=VW)
                nc.vector.tensor_copy(v3d[:, :, HD : HD + 1], ones_col)
                vps = sps_pool.tile([P, S], F32, tag="sps", name="vps")
                for kt in range(NK):
                    for hc in range(2):
                        nc.tensor.matmul(
                            vps[:, hc * 512 : (hc + 1) * 512],
                            lhsT=xt[:, kt, st * P : (st + 1) * P],
                            rhs=wv_t[:, kt, hc * 512 : (hc + 1) * 512],
                            start=(kt == 0),
                            stop=(kt == NK - 1),
                        )
                nc.vector.tensor_add(
                    v3d[:, :, 0:HD],
                    vps.rearrange("p (h c) -> p h c", c=HD),
                    bv_b.rearrange("p (h c) -> p h c", c=HD),
                )

        # ---- per head pair: project Q^T/K^T, then attention for both heads ----
        def emit_proj(hp):
            """Q^T/K^T projection for head pair hp: PE accumulation through
            the sps ring + bias add onto SBUF (bf16)."""
            qt_t = qk_pool.tile([P, S], BF16, tag="qt")
            qps = sps_pool.tile([P, S], F32, tag="sps", name="qps")
            for kt in range(NK):
                for sc in range(2):
                    nc.tensor.matmul(
                        qps[:, sc * 512 : (sc + 1) * 512],
                        lhsT=wq_t[:, kt, hp * P : (hp + 1) * P],
                        rhs=xt[:, kt, sc * 512 : (sc + 1) * 512],
                        start=(kt == 0),
                        stop=(kt == NK - 1),
                    )
            nc.vector.tensor_scalar_add(qt_t, qps, bq_t[:, hp : hp + 1])

            kt_t = qk_pool.tile([P, S], BF16, tag="kt")
            kps = sps_pool.tile([P, S], F32, tag="sps", name="kps")
            for kt in range(NK):
                for sc in range(2):
                    nc.tensor.matmul(
                        kps[:, sc * 512 : (sc + 1) * 512],
                        lhsT=wk_t[:, kt, hp * P : (hp + 1) * P],
                        rhs=xt[:, kt, sc * 512 : (sc + 1) * 512],
                        start=(kt == 0),
                        stop=(kt == NK - 1),
                    )
            nc.vector.tensor_scalar_add(kt_t, kps, bk_t[:, hp : hp + 1])
            return qt_t, kt_t

        # Project Q^T/K^T for ALL head pairs up front (a pure PE GEMM streak).
        # The attention loop's PSUM ring then only serves scores, so
        # scores(hp+1) start right after exp(hp) drains and the exp engines
        # stay hot across head-pair boundaries.
        with (
            tc.tile_pool(name="expt", bufs=4) as exp_pool,
            tc.tile_pool(name="scht", bufs=2) as sch_pool,
        ):
            n_hp = NHP if "qk" in phases else 0
            qk_all = [emit_proj(h) for h in range(n_hp)]
            for hp in range(n_hp):
                qt_t, kt_t = qk_all[hp]

                outp = outp_pool.tile([P, S], F32, tag="outp")
                a_on = "attn" in phases
                exp2 = [
                    exp_pool.tile([P, NK * S], BF16, tag="expt", name=f"exp{i}")
                    for i in range(2)
                ]
                cps2 = [
                    cps_pool.tile([P, 1024], F32, tag="cps", name=f"cps{i}")
                    for i in range(2)
                ]
                for kt in range(NK if a_on else 0):
                    sp2 = [
                        sps_pool.tile([P, S], F32, tag="sps", name=f"sp{i}")
                        for i in range(2)
                    ]
                    # sub-outer: each head's K^T block stays stationary for
                    # both query chunks (one weight load per head per kt);
                    # subs still land on distinct row groups / PSUM banks so
                    # the PE can overlap them
                    for sub in range(2):
                        r0 = HD * sub
                        for qn in range(2):
                            nc.tensor.matmul(
                                sp2[sub][:, qn * 512 : (qn + 1) * 512],
                                lhsT=kt_t[r0 : r0 + HD, kt * P : (kt + 1) * P],
                                rhs=qt_t[r0 : r0 + HD, qn * 512 : (qn + 1) * 512],
                                start=True,
                                stop=True,
                            )
                    for sub in range(2):
                        if sub == 1 and kt in dve_kts:
                            # two-point Schraudolph on DVE (see header)
                            tma = sch_pool.tile([P, S], BF16, tag="scha")
                            tmb = sch_pool.tile([P, S], BF16, tag="schb")
                            nc.vector.tensor_scalar(
                                out=tma.bitcast(I16),
                                in0=sp2[sub],
                                scalar1=SCH_A,
                                scalar2=schb_t[:, kt : kt + 1],
                                op0=mybir.AluOpType.mult,
                                op1=mybir.AluOpType.add,
                            )
                            # i16_b = i16_a - 64 exactly (int domain, 4x mode)
                            nc.vector.tensor_scalar_add(
                                tmb.bitcast(I16), tma.bitcast(I16), -64
                            )
                            nc.vector.scalar_tensor_tensor(
                                out=exp2[sub][:, kt * S : (kt + 1) * S],
                                in0=tmb,
                                scalar=SQRT2,
                                in1=tma,
                                op0=mybir.AluOpType.mult,
                                op1=mybir.AluOpType.add,
                            )
                        else:
                            nc.scalar.activation(
                                exp2[sub][:, kt * S : (kt + 1) * S],
                                sp2[sub],
                                Exp,
                                bias=mask_t[:, kt : kt + 1],
                                scale=1.0 / np.sqrt(HD),
                            )
                    for sub in range(2):
                        h = 2 * hp + sub
                        for qn in range(2):
                            nc.tensor.matmul(
                                cps2[sub][:VW, qn * 512 : (qn + 1) * 512],
                                lhsT=vp[kt][:, h * VW : (h + 1) * VW],
                                rhs=exp2[sub][
                                    :, kt * S + qn * 512 : kt * S + qn * 512 + 512
                                ],
                                start=(kt == 0),
                                stop=(kt == NK - 1),
                            )
                for sub in range(2 if a_on else 0):
                    r0 = HD * sub
                    ctx_sb = ctx_pool.tile([VW, S], F32, tag="ctx")
                    # sub0's drain copy rides ScalarE (it has slack; DVE is
                    # the attention pacer), sub1's stays on DVE
                    if sub == 0:
                        nc.scalar.copy(ctx_sb, cps2[sub][:VW, :])
                    else:
                        nc.vector.tensor_copy(ctx_sb, cps2[sub][:VW, :])
                    # batched transposes into one padded PSUM tile (per-qt
                    # stride 128 keeps each transpose inside one bank)
                    tpb = cps_pool.tile([P, NS, P], F32, tag="cps", name="tpb")
                    for qt in range(NS):
                        nc.tensor.transpose(
                            tpb[:, qt, :VW],
                            ctx_sb[:, qt * P : (qt + 1) * P],
                            ident[:VW, :VW],
                        )
                    # one batched reciprocal over all 8 denominator columns,
                    # then one broadcast multiply scattering to output layout
                    rec = rec_pool.tile([P, NS], F32, tag="rec")
                    nc.vector.reciprocal(rec, tpb[:, :, HD])
                    rec_b = bass.AP(
                        tensor=rec.tensor,
                        offset=rec.offset,
                        ap=[rec.ap[0], rec.ap[1], [0, HD]],
                    )
                    nc.vector.tensor_mul(
                        outp.rearrange("p (q c) -> p q c", c=P)[:, :, r0 : r0 + HD],
                        tpb[:, :, 0:HD],
                        rec_b,
                    )
                if "attn" in phases:
                    # issue on the idle GPSIMD DGE: these wait on the whole
                    # attention pipeline, and on the in-order SP queue they
                    # would block the next rep's input loads from issuing
                    nc.gpsimd.dma_start(
                        out=out.rearrange("(q p) c -> p q c", p=P)[
                            :, :, hp * P : (hp + 1) * P
                        ],
                        in_=outp.rearrange("p (q c) -> p q c", c=P),
                    )


def _split_excess_waits(nc):
    """This walrus build rejects instructions with more than a couple of sem
    waits. Hoist excess waits onto injected same-engine NoOps that execute
    immediately before the overfull instruction (program order per engine is
    the basic-block order, so the waits still complete first)."""
    counter = 0
    for func in nc.m.functions:
        for block in func.blocks:
            insts = block.instructions
            out = []
            changed = False
            for inst in insts:
                si = inst.sync_info
                limit = 2 if type(inst).__name__ == "InstEventSemaphore" else 1
                if si is not None and len(si.on_wait) > limit:
                    waits = list(si.on_wait)
                    for w in waits[limit:]:
                        nop = mybir.InstNoOp(
                            name=f"I-wsplit-{counter}", engine=inst.engine
                        )
                        counter += 1
                        nop.sync_info = mybir.SyncInfo(on_wait=[w], on_update=[])
                        nop.debug = inst.debug
                        out.append(nop)
                    si.on_wait = waits[:limit]
                    inst.sync_info = si
                    changed = True
                out.append(inst)
            if changed:
                block.instructions = out


_NC_CACHE = {}


def _build(split_waits=True, n_reps=1, phases=("v", "qk", "attn"), **kw):
    global _NC_CACHE
    key = (n_reps, tuple(phases), tuple(sorted(kw.items())))
    if split_waits and key in _NC_CACHE:
        return _NC_CACHE[key]
    _patch_tile_drain()
    nc = bass.Bass(target_bir_lowering=False, debug=False)
    hst = nc.dram_tensor("hst", [P, NK, S], BF16, kind="ExternalInput").ap()
    mask = nc.dram_tensor("mask", [S], F32, kind="ExternalInput").ap()
    wqf = nc.dram_tensor("wqf", [P, NK, H], BF16, kind="ExternalInput").ap()
    bq = nc.dram_tensor("bq", [H], F32, kind="ExternalInput").ap()
    wkf = nc.dram_tensor("wkf", [P, NK, H], BF16, kind="ExternalInput").ap()
    bk = nc.dram_tensor("bk", [H], F32, kind="ExternalInput").ap()
    wvf = nc.dram_tensor("wvf", [P, NK, H], BF16, kind="ExternalInput").ap()
    bv = nc.dram_tensor("bv", [H], F32, kind="ExternalInput").ap()
    out = nc.dram_tensor("out", [S, H], F32, kind="ExternalOutput").ap()
    from contextlib import ExitStack

    with tile.TileContext(nc) as tc, ExitStack() as cctx:
        consts = build_consts(nc, tc, cctx, mask, bq, bk, bv)
        for _ in range(n_reps):
            build_kernel(
                nc, tc, consts, hst, mask, wqf, bq, wkf, bk, wvf, bv, out,
                phases=phases, **kw
            )
    if not split_waits:
        return nc
    _split_excess_waits(nc)
    _NC_CACHE[key] = nc
    return nc


def make_in_maps(hidden_states, attention_mask, Wq, bq, Wk, bk, Wv, bv):
    import ml_dtypes

    bf16 = ml_dtypes.bfloat16
    f = np.ascontiguousarray

    def feed(w):
        # [H, H] -> [P, NK, H] with w[t*P+p, m] at [p, t, m]
        return f(
            np.asarray(w, dtype=np.float32)
            .reshape(NK, P, H)
            .transpose(1, 0, 2)
            .astype(bf16)
        )

    wqf, wkf, wvf = feed(Wq), feed(Wk), feed(Wv)
    maps = []
    for i in range(N_CORES):
        x = np.asarray(hidden_states[i], dtype=np.float32)
        # X^T feed: [P, NK, S] with X[s, k*P+p] at [p, k, s]
        hst = f(x.reshape(S, NK, P).transpose(2, 1, 0).astype(bf16))
        maps.append(
            {
                "hst": hst,
                "mask": f(
                    np.asarray(attention_mask[i], dtype=np.float32).reshape(S)
                ),
                "wqf": wqf,
                "bq": f(np.asarray(bq, dtype=np.float32)),
                "wkf": wkf,
                "bk": f(np.asarray(bk, dtype=np.float32)),
                "wvf": wvf,
                "bv": f(np.asarray(bv, dtype=np.float32)),
            }
        )
    return maps


def run(in_maps, **kwargs):
    nc = _build()
    return run_bass_kernel_spmd(nc, in_maps, core_ids=list(range(N_CORES)), **kwargs)


def kernel(hidden_states, attention_mask, Wq, bq, Wk, bk, Wv, bv):
    in_maps = make_in_maps(hidden_states, attention_mask, Wq, bq, Wk, bk, Wv, bv)
    res = run(in_maps)
    return np.stack([res.results[i]["out"] for i in range(N_CORES)], axis=0)
